# revision 31
# baseline (speedup 1.0000x reference)
"""Multi-head attention (16 heads, RoPE, causal) for Trainium2, 8 NeuronCores.

Sharding: data-parallel over batch (2) x tensor-parallel over head groups (4),
one (batch, head-group-of-4) pair per core. Each core computes its 4 heads'
attention feature-major (transposed) and a partial output projection
outT = Wo_slice^T @ Y^T [1024, 2048] in fp16; the host sums the 4 partials per
batch and transposes back.

Key structure (v3, pipelined):
  - Everything feature-major: Q^T/K^T [256, 2048] so the S^T (j-on-partition)
    matmuls need no on-chip transposes anywhere.
  - S^T for the two heads of a pair (rows 0:64 / 64:128 of a 128-row tile)
    is emitted interleaved so the K=64 matmuls run CONCURRENTLY in disjoint
    PE row-groups (tile_position auto-derived from base partition).
  - Projections (next l-chunk), V tiles, and the output projection of the
    previous chunk are interleaved as filler tensor work inside the attention
    jp-loop, overlapping with exp() on the scalar engine.
  - ~28 junk warm-up matmuls at t=0 keep the PE HAM clock-gate warm so the
    DMA-gated prologue projections run at 2.4 GHz instead of 1.2.
  - Output partials in fp16; RoPE tables built on-chip from 32-row cos/sin;
    input DMAs split across both HWDGE queues (sync + scalar).
  - softmax denominator comes from a ones-column appended to V (O_aug row 64)
    so no partition reductions; exp() has no max-subtraction (logits tiny for
    this problem family; host-side spectral bound checks and falls back).
"""

import sys

sys.path.insert(0, "/opt/trn_rl_repo")
sys.path.insert(0, "/root/.axon_site")

import numpy as np

B, L, D = 2, 2048, 1024
H = 16                  # total heads
HD = 64                 # head dim
HPC = 4                 # heads per core
NCORES = 8
NT = 2                  # head pairs per core (128-row tiles of Q^T/K^T/Y^T)
LC = L // 512           # 512-wide l chunks
KC = D // 128           # 128-deep contraction chunks over model dim
LT = L // 128           # 128-row l tiles

_cache = {}


def _build_nc(causal: bool, taylor: bool = False):
    import contextlib

    import concourse.bass as bass
    import concourse.tile as tile
    from concourse import bacc, mybir

    F32 = mybir.dt.float32
    F16 = mybir.dt.float16
    BF16 = mybir.dt.bfloat16
    EXP = mybir.ActivationFunctionType.Exp

    nc = bacc.Bacc("TRN2", target_bir_lowering=False, debug=False, num_devices=NCORES)

    # lc-major x layout: for a fixed (partition, lc) the (kc, 512) block is
    # 8KB contiguous in DRAM, so the per-lc DMAs move big lines (the kc-major
    # layout produced 1KB lines and halved effective DMA bandwidth).
    xb = nc.dram_tensor("xb", [128, LC, KC, 512], BF16, kind="ExternalInput")
    wq = nc.dram_tensor("wq", [128, KC * 256], BF16, kind="ExternalInput")
    wk = nc.dram_tensor("wk", [128, KC * 256], BF16, kind="ExternalInput")
    wv = nc.dram_tensor("wv", [128, KC * 256], BF16, kind="ExternalInput")
    wo = nc.dram_tensor("wo", [128, 2 * D], BF16, kind="ExternalInput")
    cs128 = nc.dram_tensor("cs128", [128, L], BF16, kind="ExternalInput")
    sn128 = nc.dram_tensor("sn128", [128, L], BF16, kind="ExternalInput")
    mk4 = nc.dram_tensor("mk4", [128, 128], BF16, kind="ExternalInput")
    outT = nc.dram_tensor("outT", [D, L], F16, kind="ExternalOutput")

    with tile.TileContext(nc) as tc, \
         nc.allow_low_precision(reason="bf16 matmul pipeline by design"), \
         contextlib.ExitStack() as ctx:
        p_w = ctx.enter_context(tc.tile_pool(name="p_w", bufs=3))
        p_wo = ctx.enter_context(tc.tile_pool(name="p_wo", bufs=1))
        p_const = ctx.enter_context(tc.tile_pool(name="p_const", bufs=4))
        p_x = ctx.enter_context(tc.tile_pool(name="p_x", bufs=5))
        p_junk = ctx.enter_context(tc.tile_pool(name="p_junk", bufs=1))
        p_qt = ctx.enter_context(tc.tile_pool(name="p_qt", bufs=2))
        p_kt = ctx.enter_context(tc.tile_pool(name="p_kt", bufs=2))
        p_yt = ctx.enter_context(tc.tile_pool(name="p_yt", bufs=2))
        p_v = ctx.enter_context(tc.tile_pool(name="p_v", bufs=16))
        p_pt = ctx.enter_context(tc.tile_pool(name="p_pt", bufs=6))
        p_tmp = ctx.enter_context(tc.tile_pool(name="p_tmp", bufs=4))
        p_z = ctx.enter_context(tc.tile_pool(name="p_z", bufs=8))
        p_oc = ctx.enter_context(tc.tile_pool(name="p_oc", bufs=4))
        pp = ctx.enter_context(tc.tile_pool(name="pp", bufs=2, space="PSUM"))
        pst = ctx.enter_context(tc.tile_pool(name="pst", bufs=2, space="PSUM"))
        pso = ctx.enter_context(tc.tile_pool(name="pso", bufs=2, space="PSUM"))

        # ---- PE warm-up + exp-table preload while DMAs run ----------
        # ~28 junk matmuls span the first ~8us so the HAM clock-gate is
        # warm (2.4 GHz) when the real, DMA-gated projections start.
        # The first 8 warm-up matmuls also ZERO all four pst banks: the
        # diagonal-trimmed S^T matmuls leave sub-regions of st unwritten,
        # and exp() of uninitialized PSUM garbage can overflow to inf (then
        # inf*0 = NaN in the causal mask multiply). The remaining short-N
        # matmuls just keep the PE HAM clock-gate warm (~10us span) until
        # the DMA-gated real projections start.
        junk = p_junk.tile([128, 512], BF16, tag="junk")
        nc.vector.memset(junk[:, :], 0.0)
        jps = [pst.tile([128, 1024], F32, tag="st", name=f"warm{i}")
               for i in range(2)]
        for r in range(8):
            t = jps[(r // 2) % 2]
            half = slice(0, 512) if r % 2 == 0 else slice(512, 1024)
            nc.tensor.matmul(t[:, half], junk[:, 0:128], junk[:, :],
                             start=True, stop=True)
        for r in range(48):
            nc.tensor.matmul(jps[r % 2][:, 0:128], junk[:, 0:128],
                             junk[:, 0:128], start=True, stop=True)

        # ---- input DMAs (split across the two HWDGE queues) ---------
        # sync queue: cos table, then x slices lc-major so the lc=0 columns
        # land first (first two kc chunks of lc=0 before the rest).
        cos_t = p_const.tile([128, L], BF16, tag="const", name="cos")
        nc.sync.dma_start(out=cos_t[:, :], in_=cs128.ap())
        xs_a = p_x.tile([128, 2, 512], BF16, tag="x", name="xa")
        nc.sync.dma_start(out=xs_a[:, :, :], in_=xb.ap()[:, 0, 0:2, :])
        xs_b = p_x.tile([128, 6, 512], BF16, tag="x", name="xb")
        nc.sync.dma_start(out=xs_b[:, :, :], in_=xb.ap()[:, 0, 2:8, :])
        xs_r = []
        for lc in range(1, LC):
            t = p_x.tile([128, KC, 512], BF16, tag="x", name=f"x{lc}")
            nc.sync.dma_start(out=t[:, :, :], in_=xb.ap()[:, lc, :, :])
            xs_r.append(t)

        def x_ap(kc, col0, ncol):
            # [128, ncol] slice of x at model chunk kc, columns col0:col0+ncol
            lc, o = col0 // 512, col0 % 512
            assert o + ncol <= 512
            if lc == 0:
                t = xs_a if kc < 2 else xs_b
                k = kc if kc < 2 else kc - 2
            else:
                t, k = xs_r[lc - 1], kc
            return t[:, k, o:o + ncol]

        # scalar queue: weights + srot table, in order of first use
        wq_sb = p_w.tile([128, KC, 256], BF16, tag="w", name="wq")
        nc.scalar.dma_start(out=wq_sb[:, :, :], in_=wq.ap())
        wk_sb = p_w.tile([128, KC, 256], BF16, tag="w", name="wk")
        nc.scalar.dma_start(out=wk_sb[:, :, :], in_=wk.ap())
        srot_t = p_const.tile([128, L], BF16, tag="const", name="srot")
        nc.scalar.dma_start(out=srot_t[:, :], in_=sn128.ap())
        wv_sb = p_w.tile([128, KC, 256], BF16, tag="w", name="wv")
        nc.scalar.dma_start(out=wv_sb[:, :, :], in_=wv.ap())
        mk_t = p_const.tile([128, 128], BF16, tag="tri")
        nc.scalar.dma_start(out=mk_t[:, :], in_=mk4.ap())
        wo_sb = p_wo.tile([128, 2, D], BF16, tag="wo")
        nc.scalar.dma_start(out=wo_sb[:, :, :], in_=wo.ap())

        # exp-table preload (~2.7us) overlaps the DMA flow
        dz = p_z.tile([1, 8], F32, tag="dz")
        nc.vector.memset(dz[:, :], 0.0)
        dz2 = p_z.tile([1, 8], F32, tag="dz2")
        nc.scalar.activation(dz2[0:1, :], dz[0:1, :], EXP)

        qt_sb = [p_qt.tile([128, L], BF16, tag="qt", name=f"qt{i}") for i in range(NT)]
        kt_sb = [p_kt.tile([128, L], BF16, tag="kt", name=f"kt{i}") for i in range(NT)]
        yt_sb = [p_yt.tile([128, L], BF16, tag="yt", name=f"yt{i}") for i in range(NT)]
        v_sb = [p_v.tile([128, HPC, 65], BF16, tag="vaug", name=f"vaug{i}")
                for i in range(LT)]

        # ---- unit emitters ------------------------------------------
        # rope: evacuate proj psum to bf16, then rotate-pairs. The four
        # narrow rotate muls go to the (otherwise idle) gpsimd engine for
        # steady-state units; prologue units keep them on the faster DVE
        # since they gate the start of attention.
        def rope_evac(ps, trg, lc, evac_act, narrows_gps):
            sl = slice(lc * 512, (lc + 1) * 512)
            qraw = p_tmp.tile([128, 512], BF16, tag="qraw")
            if evac_act:
                nc.scalar.copy(qraw[:, :], ps[:, :])
            else:
                nc.vector.tensor_copy(qraw[:, :], ps[:, :])
            eng = nc.gpsimd if narrows_gps else nc.vector
            tmp = p_tmp.tile([128, 512], BF16, tag="tmp")
            for hh in range(2):
                b0 = hh * 64
                eng.tensor_mul(tmp[b0:b0 + 32, :], qraw[b0 + 32:b0 + 64, :],
                               srot_t[b0 + 32:b0 + 64, sl])
                eng.tensor_mul(tmp[b0 + 32:b0 + 64, :], qraw[b0:b0 + 32, :],
                               srot_t[b0:b0 + 32, sl])
            nc.vector.tensor_mul(trg[:, sl], qraw[:, :], cos_t[:, sl])
            nc.vector.tensor_add(trg[:, sl], trg[:, sl], tmp[:, :])

        def proj_unit(w_sb, trg, nt, lc, evac_act=False, narrows_gps=False):
            ps = pp.tile([128, 512], F32, tag="pp")
            for kc in range(KC):
                nc.tensor.matmul(
                    ps[:, :], w_sb[:, kc, nt * 128:(nt + 1) * 128],
                    x_ap(kc, lc * 512, 512),
                    start=(kc == 0), stop=(kc == KC - 1))
            rope_evac(ps, trg, lc, evac_act, narrows_gps)

        def v_unit(lt, evac_act=False):
            ps = pp.tile([128, 256], F32, tag="pp")
            for kc in range(KC):
                nc.tensor.matmul(
                    ps[:, :], x_ap(kc, lt * 128, 128),
                    wv_sb[:, kc, :], start=(kc == 0), stop=(kc == KC - 1))
            va = v_sb[lt]
            nc.vector.memset(va[:, :, 64:65], 1.0)
            copy = nc.scalar.copy if evac_act else nc.vector.tensor_copy
            copy(va[:, :, 0:64], ps[:, :].rearrange("p (h v) -> p h v", h=HPC))

        def oproj_unit(c, ot, use_act):
            csl = slice(c * 512, (c + 1) * 512)
            ps = pp.tile([128, 512], F32, tag="pp")
            for kc2 in range(2):
                nc.tensor.matmul(
                    ps[:, :], wo_sb[:, kc2, ot * 128:(ot + 1) * 128],
                    yt_sb[kc2][:, csl], start=(kc2 == 0), stop=(kc2 == 1))
            oc = p_oc.tile([128, 512], F16, tag="oc")
            if use_act:
                nc.scalar.copy(oc[:, :], ps[:, :])
            else:
                nc.vector.tensor_copy(oc[:, :], ps[:, :])
            nc.sync.dma_start(
                out=outT.ap()[ot * 128:(ot + 1) * 128, csl], in_=oc[:, :])

        # ---- attention ----------------------------------------------
        def trim(j, c):
            k = j - 4 * c
            return 128 * k if (causal and k >= 0) else 0

        MULT = mybir.AluOpType.mult
        ADD = mybir.AluOpType.add

        def taylor_exp(pt, st):
            # exp(x) ~ 1 + x(1 + x/2(1 + x/3)) on the vector engine, used
            # for a few chunk-3 groups where the scalar engine saturates.
            # Valid because the host checked max|logit| is small.
            cb = p_tmp.tile([128, 1024], BF16, tag="tb")
            nc.vector.tensor_copy(cb[:, :], st[:, :])
            a = p_tmp.tile([128, 1024], BF16, tag="ta")
            nc.vector.tensor_scalar(a[:, :], cb[:, :], 1.0 / 3.0, 1.0, MULT, ADD)
            b = p_tmp.tile([128, 1024], BF16, tag="ta")
            nc.vector.scalar_tensor_tensor(b[:, :], a[:, :], 0.5, cb[:, :], MULT, MULT)
            c2 = p_tmp.tile([128, 1024], BF16, tag="ta")
            nc.vector.scalar_tensor_tensor(c2[:, :], b[:, :], 1.0, cb[:, :], ADD, MULT)
            nc.vector.tensor_scalar(pt[:, :], c2[:, :], 1.0, None, ADD)

        def emit_s(c, nt, jp):
            stA = pst.tile([128, 1024], F32, tag="st", name="stA")
            stB = pst.tile([128, 1024], F32, tag="st", name="stB")
            for s in range(2):
                j = 2 * jp + s
                # s=0 may trim its leading columns (exp starts there too);
                # s=1 writes its full 512 so exp never reads stale PSUM.
                t = trim(j, c) if s == 0 else 0
                for st, r0 in ((stA, 0), (stB, 64)):
                    nc.tensor.matmul(
                        st[:, s * 512 + t:(s + 1) * 512],
                        kt_sb[nt][r0:r0 + 64, j * 128:(j + 1) * 128],
                        qt_sb[nt][r0:r0 + 64, c * 512 + t:(c + 1) * 512],
                        start=True, stop=True)
            t0 = trim(2 * jp, c)
            pts = []
            for i, st in enumerate((stA, stB)):
                pt = p_pt.tile([128, 1024], BF16, tag="pt")
                if taylor and i == 1 and c == LC - 1 and jp in (1, 3, 5):
                    taylor_exp(pt, st)      # non-diagonal groups only (t0=0)
                else:
                    nc.scalar.activation(pt[:, t0:], st[:, t0:], EXP)
                pts.append(pt)
            if causal:
                for s in range(2):
                    k = 2 * jp + s - 4 * c
                    if k >= 0:
                        sl = slice(s * 512 + 128 * k, s * 512 + 128 * (k + 1))
                        for pt in pts:
                            nc.vector.tensor_mul(pt[:, sl], pt[:, sl], mk_t[:, :])
            return pts

        def emit_pv(c, nt, jp, pts, oaugs, jmax):
            for s in range(2):
                j = 2 * jp + s
                t = trim(j, c)
                for i, (pt, oaug) in enumerate(zip(pts, oaugs)):
                    h = 2 * nt + i
                    nc.tensor.matmul(
                        oaug[:, t:512], v_sb[j][:, h, :],
                        pt[:, s * 512 + t:(s + 1) * 512],
                        start=(j == 0), stop=(j == jmax))

        def emit_norm(c, nt, oaugs):
            csl = slice(c * 512, (c + 1) * 512)
            for i, oaug in enumerate(oaugs):
                r0 = i * 64
                # stage z in SBUF first: reciprocal_approx_fast is a
                # multi-pass custom DVE op; feeding it PSUM directly has
                # produced corrupted single columns on hardware.
                zs = p_z.tile([1, 512], F32, tag="zs")
                nc.vector.tensor_copy(zs[0:1, :], oaug[64:65, :])
                zrow = p_z.tile([1, 512], F32, tag="zrow")
                nc.vector.reciprocal_approx_fast(zrow[0:1, :], zs[0:1, :])
                zb = p_z.tile([64, 512], F32, tag="zb")
                nc.gpsimd.partition_broadcast(zb[:, :], zrow[0:1, :])
                nc.vector.tensor_mul(yt_sb[nt][r0:r0 + 64, csl],
                                     oaug[0:64, :], zb[:, :])

        # ---- prologue: first chunk of projections + first V tiles ---
        # (dense mode emits all V tiles up front: PV(c=0) touches every j)
        # evac copies on the idle scalar engine, rotate muls on DVE: the
        # vector-engine chain here gates the start of attention.
        for nt in range(NT):
            proj_unit(wq_sb, qt_sb[nt], nt, 0, evac_act=True, narrows_gps=False)
            proj_unit(wk_sb, kt_sb[nt], nt, 0, evac_act=True, narrows_gps=False)
        for lt in range(4 if causal else LT):
            v_unit(lt, evac_act=True)

        # ---- main pipelined loop ------------------------------------
        for c in range(LC):
            jmax = 4 * c + 3 if causal else LT - 1
            npj = (jmax + 1) // 2            # jp count per pair

            # filler units to interleave into this chunk's attention.
            # out-projections are deferred toward the last chunk, which is
            # otherwise exp-bound (its slots have spare tensor time).
            fills = []
            if c < LC - 1:
                for nt in range(NT):
                    fills.append(lambda nt=nt: proj_unit(wq_sb, qt_sb[nt], nt, c + 1,
                                                         evac_act=True))
                    fills.append(lambda nt=nt: proj_unit(wk_sb, kt_sb[nt], nt, c + 1,
                                                         evac_act=True))
                if causal:
                    for lt in range(4 * (c + 1), 4 * (c + 2)):
                        fills.append(lambda lt=lt: v_unit(lt, evac_act=True))
            for cp in {2: [0], 3: [1, 2]}.get(c, []):
                for ot in range(8):
                    fills.append(lambda ot=ot, cp=cp: oproj_unit(cp, ot, False))

            nslots = 2 * npj
            state = {"fi": 0}

            def emit_fills(progress, fills=fills, state=state, nslots=nslots):
                want = min(len(fills), (len(fills) * progress) // (2 * nslots))
                while state["fi"] < want:
                    fills[state["fi"]]()
                    state["fi"] += 1

            slot = 0
            for nt in range(NT):
                oaugs = [pso.tile([65, 512], F32, tag="oaug", name=f"oaug{i}")
                         for i in range(2)]
                lag = []
                for jp in range(npj):
                    # fills first: their psum-evac copies land on the ACT/DVE
                    # queues ahead of this slot's exps, releasing pp sooner
                    emit_fills(2 * slot + 1)
                    pts = emit_s(c, nt, jp)
                    if lag:
                        emit_pv(c, nt, *lag.pop(0), oaugs, jmax)
                    slot += 1
                    emit_fills(2 * slot)
                    lag.append((jp, pts))
                for jp, pts in lag:
                    emit_pv(c, nt, jp, pts, oaugs, jmax)
                emit_norm(c, nt, oaugs)
            emit_fills(2 * nslots)

        # ---- tail: output projection of the last chunk --------------
        for ot in range(8):
            oproj_unit(LC - 1, ot, ot % 2 == 1)

    nc.compile()
    return nc


def _get_nc(causal: bool, taylor: bool = False):
    key = ("causal" if causal else "dense") + ("_taylor" if taylor else "")
    if key not in _cache:
        _cache[key] = _build_nc(causal, taylor)
    return _cache[key]


def _rope_np(x):
    d, s = x.shape[-1], x.shape[-2]
    ts = np.arange(0, d, 2, dtype=np.float32)
    inv = 10000.0 ** (-ts / d)
    grid = np.arange(s, dtype=np.float32)[:, None] * inv[None, :]
    sin = np.repeat(np.sin(grid), 2, axis=-1)
    cos = np.repeat(np.cos(grid), 2, axis=-1)
    x1, x2 = x[..., ::2], x[..., 1::2]
    xs = np.stack([-x2, x1], axis=-1).reshape(x.shape)
    return x * cos + xs * sin


def _reference_np(x, mask, Wq, Wk, Wv, Wo):
    b, l, d = x.shape
    h, k_sz = H, D // H
    split = lambda t: t.reshape(b, l, h, k_sz).transpose(0, 2, 1, 3)
    q = split((x @ Wq) / np.sqrt(np.float32(d)))
    q = _rope_np(q)
    k = _rope_np(split(x @ Wk))
    v = split(x @ Wv)
    logits = np.einsum("bhik,bhjk->bhij", q, k) + mask
    m = logits.max(axis=-1, keepdims=True)
    p = np.exp(logits - m)
    a = p / p.sum(axis=-1, keepdims=True)
    y = np.einsum("bhij,bhjv->bhiv", a, v)
    y = y.transpose(0, 2, 1, 3).reshape(b, l, d)
    return (y @ Wo).astype(np.float32)


def _spectral_norm(w, iters=12):
    rng = np.random.default_rng(0)
    v = rng.standard_normal(w.shape[1]).astype(np.float32)
    for _ in range(iters):
        u = w @ v
        u /= (np.linalg.norm(u) + 1e-30)
        v = w.T @ u
        nv = np.linalg.norm(v)
        v /= (nv + 1e-30)
    return float(nv)


def _host_consts():
    inv = 10000.0 ** (-np.arange(0, HD, 2, dtype=np.float32) / HD)
    grid = np.arange(L, dtype=np.float32)[None, :] * inv[:, None]   # [32, L]
    cos32 = np.cos(grid).astype(np.float32)
    sin32 = np.sin(grid).astype(np.float32)
    cos128 = np.ascontiguousarray(np.tile(cos32, (4, 1)))
    # srot rows r: +sin[r%32] for r%64 < 32, -sin[r%32] otherwise
    srot128 = np.ascontiguousarray(
        np.tile(np.concatenate([sin32, -sin32], axis=0), (2, 1)))
    tri = (np.arange(128)[None, :] >= np.arange(128)[:, None]).astype(np.float32)
    return cos128, srot128, np.ascontiguousarray(tri)


def _make_in_maps(x, Wq, Wk, Wv, Wo):
    import ml_dtypes
    bf16 = ml_dtypes.bfloat16

    cos128, srot128, mk4 = _host_consts()
    cos128 = cos128.astype(bf16)
    srot128 = srot128.astype(bf16)
    mk4 = mk4.astype(bf16)
    perm = np.concatenate([np.arange(0, 64, 2), np.arange(1, 64, 2)])
    Wq_s = (Wq / np.sqrt(np.float32(D))).astype(np.float32)

    def chunked(w):
        # [D, 256] -> [128, KC*256] with model-dim chunk kc in the free dim
        return np.ascontiguousarray(
            w.reshape(KC, 128, 256).transpose(1, 0, 2).reshape(128, KC * 256))

    in_maps = []
    for core in range(NCORES):
        bi, g = core // 4, core % 4
        # x feature-major, lc-major: [128, LC, KC, 512], d = kc*128 + p
        xT_b = np.ascontiguousarray(
            x[bi].T.reshape(KC, 128, LC, 512).transpose(1, 2, 0, 3)).astype(bf16)
        wq_c = np.empty((D, 256), np.float32)
        wk_c = np.empty((D, 256), np.float32)
        for hh in range(HPC):
            h_abs = g * HPC + hh
            wq_c[:, hh * 64:(hh + 1) * 64] = Wq_s[:, h_abs * 64:(h_abs + 1) * 64][:, perm]
            wk_c[:, hh * 64:(hh + 1) * 64] = Wk[:, h_abs * 64:(h_abs + 1) * 64][:, perm]
        wo_c = Wo[g * 256:(g + 1) * 256, :]        # [256, D]
        wo_c = np.ascontiguousarray(
            wo_c.reshape(2, 128, D).transpose(1, 0, 2).reshape(128, 2 * D))
        in_maps.append({
            "xb": xT_b,
            "wq": chunked(wq_c).astype(bf16),
            "wk": chunked(wk_c).astype(bf16),
            "wv": chunked(Wv[:, g * 256:(g + 1) * 256].astype(np.float32)).astype(bf16),
            "wo": wo_c.astype(bf16),
            "cs128": cos128, "sn128": srot128, "mk4": mk4,
        })
    return in_maps


def kernel(x, mask, Wq, Wk, Wv, Wo):
    from concourse.bass_utils import run_bass_kernel_spmd

    x = np.asarray(x, dtype=np.float32)
    mask = np.asarray(mask, dtype=np.float32)
    Wq = np.asarray(Wq, dtype=np.float32)
    Wk = np.asarray(Wk, dtype=np.float32)
    Wv = np.asarray(Wv, dtype=np.float32)
    Wo = np.asarray(Wo, dtype=np.float32)

    # classify the mask
    m = mask.reshape(L, L)
    tril = np.tril(np.ones((L, L), dtype=bool))
    visible = m > -1e6
    if np.array_equal(visible, tril) and not m[tril].any():
        causal = True
    elif not m.any():
        causal = False
    else:
        return _reference_np(x, mask, Wq, Wk, Wv, Wo)

    # exact logit bound (rope preserves pair norms): guards the
    # no-max-subtraction softmax, and enables the vector-engine Taylor-exp
    # path for a few chunk-3 groups when logits are provably tiny
    q = x.reshape(-1, D) @ (Wq / np.sqrt(np.float32(D)))
    k = x.reshape(-1, D) @ Wk
    qn = np.sqrt((q.reshape(-1, H, HD) ** 2).sum(-1)).max(0)
    kn = np.sqrt((k.reshape(-1, H, HD) ** 2).sum(-1)).max(0)
    bound = float((qn * kn).max())
    if bound > 60.0:
        return _reference_np(x, mask, Wq, Wk, Wv, Wo)
    taylor = causal and bound < 0.5

    in_maps = _make_in_maps(x, Wq, Wk, Wv, Wo)
    nc = _get_nc(causal, taylor)
    res = run_bass_kernel_spmd(nc, in_maps, core_ids=list(range(NCORES)))

    out = np.empty((B, L, D), dtype=np.float32)
    for bi in range(B):
        acc = res.results[bi * 4]["outT"].astype(np.float32)
        for g in range(1, 4):
            acc += res.results[bi * 4 + g]["outT"].astype(np.float32)
        out[bi] = acc.T
    return out


# revision 32
# speedup vs baseline: 1.1859x; 1.1859x over previous
"""Multi-head attention (16 heads, RoPE, causal) for Trainium2, 8 NeuronCores.

Sharding: data-parallel over batch (2) x tensor-parallel over head groups (4),
one (batch, head-group-of-4) pair per core. Each core computes its 4 heads'
attention feature-major (transposed) and a partial output projection
outT = Wo_slice^T @ Y^T [1024, 2048] in fp16; the host sums the 4 partials per
batch and transposes back.

Key structure (v3, pipelined):
  - Everything feature-major: Q^T/K^T [256, 2048] so the S^T (j-on-partition)
    matmuls need no on-chip transposes anywhere.
  - S^T for the two heads of a pair (rows 0:64 / 64:128 of a 128-row tile)
    is emitted interleaved so the K=64 matmuls run CONCURRENTLY in disjoint
    PE row-groups (tile_position auto-derived from base partition).
  - Projections (next l-chunk), V tiles, and the output projection of the
    previous chunk are interleaved as filler tensor work inside the attention
    jp-loop, overlapping with exp() on the scalar engine.
  - ~28 junk warm-up matmuls at t=0 keep the PE HAM clock-gate warm so the
    DMA-gated prologue projections run at 2.4 GHz instead of 1.2.
  - Output partials in fp16; RoPE tables built on-chip from 32-row cos/sin;
    input DMAs split across both HWDGE queues (sync + scalar).
  - softmax denominator comes from a ones-column appended to V (O_aug row 64)
    so no partition reductions; exp() has no max-subtraction (logits tiny for
    this problem family; host-side spectral bound checks and falls back).
"""

import sys

sys.path.insert(0, "/opt/trn_rl_repo")
sys.path.insert(0, "/root/.axon_site")

import numpy as np

B, L, D = 2, 2048, 1024
H = 16                  # total heads
HD = 64                 # head dim
HPC = 4                 # heads per core
NCORES = 8
NT = 2                  # head pairs per core (128-row tiles of Q^T/K^T/Y^T)
LC = L // 512           # 512-wide l chunks
KC = D // 128           # 128-deep contraction chunks over model dim
LT = L // 128           # 128-row l tiles

_cache = {}


def _build_nc(causal: bool, taylor: bool = False):
    import contextlib

    import concourse.bass as bass
    import concourse.tile as tile
    from concourse import bacc, mybir

    F32 = mybir.dt.float32
    F16 = mybir.dt.float16
    BF16 = mybir.dt.bfloat16
    EXP = mybir.ActivationFunctionType.Exp

    nc = bacc.Bacc("TRN2", target_bir_lowering=False, debug=False, num_devices=NCORES)

    # lc-major x layout: for a fixed (partition, lc) the (kc, 512) block is
    # 8KB contiguous in DRAM, so the per-lc DMAs move big lines (the kc-major
    # layout produced 1KB lines and halved effective DMA bandwidth).
    xb = nc.dram_tensor("xb", [128, LC, KC, 512], BF16, kind="ExternalInput")
    wq = nc.dram_tensor("wq", [128, KC * 256], BF16, kind="ExternalInput")
    wk = nc.dram_tensor("wk", [128, KC * 256], BF16, kind="ExternalInput")
    wv = nc.dram_tensor("wv", [128, KC * 256], BF16, kind="ExternalInput")
    wo = nc.dram_tensor("wo", [128, 2 * D], BF16, kind="ExternalInput")
    cs128 = nc.dram_tensor("cs128", [128, L], BF16, kind="ExternalInput")
    sn128 = nc.dram_tensor("sn128", [128, L], BF16, kind="ExternalInput")
    mk4 = nc.dram_tensor("mk4", [128, 128], BF16, kind="ExternalInput")
    outT = nc.dram_tensor("outT", [D, L], F16, kind="ExternalOutput")

    with tile.TileContext(nc) as tc, \
         nc.allow_low_precision(reason="bf16 matmul pipeline by design"), \
         contextlib.ExitStack() as ctx:
        p_w = ctx.enter_context(tc.tile_pool(name="p_w", bufs=3))
        p_wo = ctx.enter_context(tc.tile_pool(name="p_wo", bufs=1))
        p_const = ctx.enter_context(tc.tile_pool(name="p_const", bufs=4))
        p_x = ctx.enter_context(tc.tile_pool(name="p_x", bufs=5))
        p_junk = ctx.enter_context(tc.tile_pool(name="p_junk", bufs=1))
        p_qt = ctx.enter_context(tc.tile_pool(name="p_qt", bufs=2))
        p_kt = ctx.enter_context(tc.tile_pool(name="p_kt", bufs=2))
        p_yt = ctx.enter_context(tc.tile_pool(name="p_yt", bufs=2))
        p_v = ctx.enter_context(tc.tile_pool(name="p_v", bufs=16))
        p_pt = ctx.enter_context(tc.tile_pool(name="p_pt", bufs=6))
        p_tmp = ctx.enter_context(tc.tile_pool(name="p_tmp", bufs=4))
        p_z = ctx.enter_context(tc.tile_pool(name="p_z", bufs=8))
        p_oc = ctx.enter_context(tc.tile_pool(name="p_oc", bufs=4))
        pp = ctx.enter_context(tc.tile_pool(name="pp", bufs=2, space="PSUM"))
        pst = ctx.enter_context(tc.tile_pool(name="pst", bufs=2, space="PSUM"))
        pso = ctx.enter_context(tc.tile_pool(name="pso", bufs=2, space="PSUM"))

        # ---- PE warm-up + exp-table preload while DMAs run ----------
        # ~28 junk matmuls span the first ~8us so the HAM clock-gate is
        # warm (2.4 GHz) when the real, DMA-gated projections start.
        # The first 8 warm-up matmuls also ZERO all four pst banks: the
        # diagonal-trimmed S^T matmuls leave sub-regions of st unwritten,
        # and exp() of uninitialized PSUM garbage can overflow to inf (then
        # inf*0 = NaN in the causal mask multiply). The remaining short-N
        # matmuls just keep the PE HAM clock-gate warm (~10us span) until
        # the DMA-gated real projections start.
        junk = p_junk.tile([128, 512], BF16, tag="junk")
        nc.vector.memset(junk[:, :], 0.0)
        jps = [pst.tile([128, 1024], F32, tag="st", name=f"warm{i}")
               for i in range(2)]
        for r in range(8):
            t = jps[(r // 2) % 2]
            half = slice(0, 512) if r % 2 == 0 else slice(512, 1024)
            nc.tensor.matmul(t[:, half], junk[:, 0:128], junk[:, :],
                             start=True, stop=True)
        for r in range(48):
            nc.tensor.matmul(jps[r % 2][:, 0:128], junk[:, 0:128],
                             junk[:, 0:128], start=True, stop=True)

        # ---- input DMAs (split across the two HWDGE queues) ---------
        # sync queue: cos table, then x slices lc-major so the lc=0 columns
        # land first (first two kc chunks of lc=0 before the rest).
        cos_t = p_const.tile([128, L], BF16, tag="const", name="cos")
        nc.sync.dma_start(out=cos_t[:, :], in_=cs128.ap())
        xs_a = p_x.tile([128, 2, 512], BF16, tag="x", name="xa")
        nc.sync.dma_start(out=xs_a[:, :, :], in_=xb.ap()[:, 0, 0:2, :])
        xs_b = p_x.tile([128, 6, 512], BF16, tag="x", name="xb")
        nc.sync.dma_start(out=xs_b[:, :, :], in_=xb.ap()[:, 0, 2:8, :])
        xs_r = []
        for lc in range(1, LC):
            t = p_x.tile([128, KC, 512], BF16, tag="x", name=f"x{lc}")
            nc.sync.dma_start(out=t[:, :, :], in_=xb.ap()[:, lc, :, :])
            xs_r.append(t)

        def x_ap(kc, col0, ncol):
            # [128, ncol] slice of x at model chunk kc, columns col0:col0+ncol
            lc, o = col0 // 512, col0 % 512
            assert o + ncol <= 512
            if lc == 0:
                t = xs_a if kc < 2 else xs_b
                k = kc if kc < 2 else kc - 2
            else:
                t, k = xs_r[lc - 1], kc
            return t[:, k, o:o + ncol]

        # scalar queue: weights + srot table, in order of first use
        wq_sb = p_w.tile([128, KC, 256], BF16, tag="w", name="wq")
        nc.scalar.dma_start(out=wq_sb[:, :, :], in_=wq.ap())
        wk_sb = p_w.tile([128, KC, 256], BF16, tag="w", name="wk")
        nc.scalar.dma_start(out=wk_sb[:, :, :], in_=wk.ap())
        srot_t = p_const.tile([128, L], BF16, tag="const", name="srot")
        nc.scalar.dma_start(out=srot_t[:, :], in_=sn128.ap())
        wv_sb = p_w.tile([128, KC, 256], BF16, tag="w", name="wv")
        nc.scalar.dma_start(out=wv_sb[:, :, :], in_=wv.ap())
        mk_t = p_const.tile([128, 128], BF16, tag="tri")
        nc.scalar.dma_start(out=mk_t[:, :], in_=mk4.ap())
        wo_sb = p_wo.tile([128, 2, D], BF16, tag="wo")
        nc.scalar.dma_start(out=wo_sb[:, :, :], in_=wo.ap())

        # exp-table preload (~2.7us) overlaps the DMA flow
        dz = p_z.tile([1, 8], F32, tag="dz")
        nc.vector.memset(dz[:, :], 0.0)
        dz2 = p_z.tile([1, 8], F32, tag="dz2")
        nc.scalar.activation(dz2[0:1, :], dz[0:1, :], EXP)

        qt_sb = [p_qt.tile([128, L], BF16, tag="qt", name=f"qt{i}") for i in range(NT)]
        kt_sb = [p_kt.tile([128, L], BF16, tag="kt", name=f"kt{i}") for i in range(NT)]
        yt_sb = [p_yt.tile([128, L], BF16, tag="yt", name=f"yt{i}") for i in range(NT)]
        v_sb = [p_v.tile([128, HPC, 65], BF16, tag="vaug", name=f"vaug{i}")
                for i in range(LT)]

        # ---- unit emitters ------------------------------------------
        # rope: evacuate proj psum to bf16, then rotate-pairs. The four
        # narrow rotate muls go to the (otherwise idle) gpsimd engine for
        # steady-state units; prologue units keep them on the faster DVE
        # since they gate the start of attention.
        def rope_evac(ps, trg, lc, evac_act, narrows_gps):
            sl = slice(lc * 512, (lc + 1) * 512)
            qraw = p_tmp.tile([128, 512], BF16, tag="qraw")
            if evac_act:
                nc.scalar.copy(qraw[:, :], ps[:, :])
            else:
                nc.vector.tensor_copy(qraw[:, :], ps[:, :])
            eng = nc.gpsimd if narrows_gps else nc.vector
            tmp = p_tmp.tile([128, 512], BF16, tag="tmp")
            for hh in range(2):
                b0 = hh * 64
                eng.tensor_mul(tmp[b0:b0 + 32, :], qraw[b0 + 32:b0 + 64, :],
                               srot_t[b0 + 32:b0 + 64, sl])
                eng.tensor_mul(tmp[b0 + 32:b0 + 64, :], qraw[b0:b0 + 32, :],
                               srot_t[b0:b0 + 32, sl])
            nc.vector.tensor_mul(trg[:, sl], qraw[:, :], cos_t[:, sl])
            nc.vector.tensor_add(trg[:, sl], trg[:, sl], tmp[:, :])

        def proj_unit(w_sb, trg, nt, lc, evac_act=False, narrows_gps=False):
            ps = pp.tile([128, 512], F32, tag="pp")
            for kc in range(KC):
                nc.tensor.matmul(
                    ps[:, :], w_sb[:, kc, nt * 128:(nt + 1) * 128],
                    x_ap(kc, lc * 512, 512),
                    start=(kc == 0), stop=(kc == KC - 1))
            rope_evac(ps, trg, lc, evac_act, narrows_gps)

        def v_unit(lt, evac_act=False):
            ps = pp.tile([128, 256], F32, tag="pp")
            for kc in range(KC):
                nc.tensor.matmul(
                    ps[:, :], x_ap(kc, lt * 128, 128),
                    wv_sb[:, kc, :], start=(kc == 0), stop=(kc == KC - 1))
            va = v_sb[lt]
            nc.vector.memset(va[:, :, 64:65], 1.0)
            copy = nc.scalar.copy if evac_act else nc.vector.tensor_copy
            copy(va[:, :, 0:64], ps[:, :].rearrange("p (h v) -> p h v", h=HPC))

        def oproj_unit(c, ot, use_act):
            csl = slice(c * 512, (c + 1) * 512)
            ps = pp.tile([128, 512], F32, tag="pp")
            for kc2 in range(2):
                nc.tensor.matmul(
                    ps[:, :], wo_sb[:, kc2, ot * 128:(ot + 1) * 128],
                    yt_sb[kc2][:, csl], start=(kc2 == 0), stop=(kc2 == 1))
            oc = p_oc.tile([128, 512], F16, tag="oc")
            if use_act:
                nc.scalar.copy(oc[:, :], ps[:, :])
            else:
                nc.vector.tensor_copy(oc[:, :], ps[:, :])
            nc.sync.dma_start(
                out=outT.ap()[ot * 128:(ot + 1) * 128, csl], in_=oc[:, :])

        # ---- attention ----------------------------------------------
        def trim(j, c):
            k = j - 4 * c
            return 128 * k if (causal and k >= 0) else 0

        MULT = mybir.AluOpType.mult
        ADD = mybir.AluOpType.add

        def taylor_exp(pt, st):
            # exp(x) ~ 1 + x(1 + x/2(1 + x/3)) on the vector engine, used
            # for a few chunk-3 groups where the scalar engine saturates.
            # Valid because the host checked max|logit| is small.
            cb = p_tmp.tile([128, 1024], BF16, tag="tb")
            nc.vector.tensor_copy(cb[:, :], st[:, :])
            a = p_tmp.tile([128, 1024], BF16, tag="ta")
            nc.vector.tensor_scalar(a[:, :], cb[:, :], 1.0 / 3.0, 1.0, MULT, ADD)
            b = p_tmp.tile([128, 1024], BF16, tag="ta")
            nc.vector.scalar_tensor_tensor(b[:, :], a[:, :], 0.5, cb[:, :], MULT, MULT)
            c2 = p_tmp.tile([128, 1024], BF16, tag="ta")
            nc.vector.scalar_tensor_tensor(c2[:, :], b[:, :], 1.0, cb[:, :], ADD, MULT)
            nc.vector.tensor_scalar(pt[:, :], c2[:, :], 1.0, None, ADD)

        def emit_s(c, nt, jp):
            stA = pst.tile([128, 1024], F32, tag="st", name="stA")
            stB = pst.tile([128, 1024], F32, tag="st", name="stB")
            for s in range(2):
                j = 2 * jp + s
                # s=0 may trim its leading columns (exp starts there too);
                # s=1 writes its full 512 so exp never reads stale PSUM.
                t = trim(j, c) if s == 0 else 0
                for st, r0 in ((stA, 0), (stB, 64)):
                    nc.tensor.matmul(
                        st[:, s * 512 + t:(s + 1) * 512],
                        kt_sb[nt][r0:r0 + 64, j * 128:(j + 1) * 128],
                        qt_sb[nt][r0:r0 + 64, c * 512 + t:(c + 1) * 512],
                        start=True, stop=True)
            t0 = trim(2 * jp, c)
            pts = []
            for i, st in enumerate((stA, stB)):
                pt = p_pt.tile([128, 1024], BF16, tag="pt")
                if taylor and i == 1 and c == LC - 1 and jp in (1, 3, 5):
                    taylor_exp(pt, st)      # non-diagonal groups only (t0=0)
                else:
                    nc.scalar.activation(pt[:, t0:], st[:, t0:], EXP)
                pts.append(pt)
            if causal:
                for s in range(2):
                    k = 2 * jp + s - 4 * c
                    if k >= 0:
                        sl = slice(s * 512 + 128 * k, s * 512 + 128 * (k + 1))
                        for pt in pts:
                            nc.vector.tensor_mul(pt[:, sl], pt[:, sl], mk_t[:, :])
            return pts

        def emit_pv(c, nt, jp, pts, oaugs, jmax):
            for s in range(2):
                j = 2 * jp + s
                t = trim(j, c)
                for i, (pt, oaug) in enumerate(zip(pts, oaugs)):
                    h = 2 * nt + i
                    nc.tensor.matmul(
                        oaug[:, t:512], v_sb[j][:, h, :],
                        pt[:, s * 512 + t:(s + 1) * 512],
                        start=(j == 0), stop=(j == jmax))

        def emit_norm(c, nt, oaugs):
            csl = slice(c * 512, (c + 1) * 512)
            for i, oaug in enumerate(oaugs):
                r0 = i * 64
                # stage z in SBUF first: reciprocal_approx_fast is a
                # multi-pass custom DVE op; feeding it PSUM directly has
                # produced corrupted single columns on hardware.
                zs = p_z.tile([1, 512], F32, tag="zs")
                nc.vector.tensor_copy(zs[0:1, :], oaug[64:65, :])
                zrow = p_z.tile([1, 512], F32, tag="zrow")
                nc.vector.reciprocal_approx_fast(zrow[0:1, :], zs[0:1, :])
                zb = p_z.tile([64, 512], F32, tag="zb")
                nc.gpsimd.partition_broadcast(zb[:, :], zrow[0:1, :])
                nc.vector.tensor_mul(yt_sb[nt][r0:r0 + 64, csl],
                                     oaug[0:64, :], zb[:, :])

        # ---- prologue: first chunk of projections + first V tiles ---
        # (dense mode emits all V tiles up front: PV(c=0) touches every j)
        # evac copies on the idle scalar engine, rotate muls on DVE: the
        # vector-engine chain here gates the start of attention.
        for nt in range(NT):
            proj_unit(wq_sb, qt_sb[nt], nt, 0, evac_act=True, narrows_gps=False)
            proj_unit(wk_sb, kt_sb[nt], nt, 0, evac_act=True, narrows_gps=False)
        for lt in range(4 if causal else LT):
            v_unit(lt, evac_act=True)

        # ---- main pipelined loop ------------------------------------
        for c in range(LC):
            jmax = 4 * c + 3 if causal else LT - 1
            npj = (jmax + 1) // 2            # jp count per pair

            # filler units to interleave into this chunk's attention.
            # out-projections are deferred toward the last chunk, which is
            # otherwise exp-bound (its slots have spare tensor time).
            fills = []
            if c < LC - 1:
                for nt in range(NT):
                    fills.append(lambda nt=nt: proj_unit(wq_sb, qt_sb[nt], nt, c + 1,
                                                         evac_act=True))
                    fills.append(lambda nt=nt: proj_unit(wk_sb, kt_sb[nt], nt, c + 1,
                                                         evac_act=True))
                if causal:
                    for lt in range(4 * (c + 1), 4 * (c + 2)):
                        fills.append(lambda lt=lt: v_unit(lt, evac_act=True))
            for cp in {2: [0], 3: [1, 2]}.get(c, []):
                for ot in range(8):
                    fills.append(lambda ot=ot, cp=cp: oproj_unit(cp, ot, False))

            nslots = 2 * npj
            state = {"fi": 0}

            def emit_fills(progress, fills=fills, state=state, nslots=nslots):
                want = min(len(fills), (len(fills) * progress) // (2 * nslots))
                while state["fi"] < want:
                    fills[state["fi"]]()
                    state["fi"] += 1

            slot = 0
            for nt in range(NT):
                oaugs = [pso.tile([65, 512], F32, tag="oaug", name=f"oaug{i}")
                         for i in range(2)]
                lag = []
                for jp in range(npj):
                    # fills first: their psum-evac copies land on the ACT/DVE
                    # queues ahead of this slot's exps, releasing pp sooner
                    emit_fills(2 * slot + 1)
                    pts = emit_s(c, nt, jp)
                    if lag:
                        emit_pv(c, nt, *lag.pop(0), oaugs, jmax)
                    slot += 1
                    emit_fills(2 * slot)
                    lag.append((jp, pts))
                for jp, pts in lag:
                    emit_pv(c, nt, jp, pts, oaugs, jmax)
                emit_norm(c, nt, oaugs)
            emit_fills(2 * nslots)

        # ---- tail: output projection of the last chunk --------------
        for ot in range(8):
            oproj_unit(LC - 1, ot, ot % 2 == 1)

    nc.compile()
    return nc


def _get_nc(causal: bool, taylor: bool = False):
    key = ("causal" if causal else "dense") + ("_taylor" if taylor else "")
    if key not in _cache:
        _cache[key] = _build_nc(causal, taylor)
    return _cache[key]


def _rope_np(x):
    d, s = x.shape[-1], x.shape[-2]
    ts = np.arange(0, d, 2, dtype=np.float32)
    inv = 10000.0 ** (-ts / d)
    grid = np.arange(s, dtype=np.float32)[:, None] * inv[None, :]
    sin = np.repeat(np.sin(grid), 2, axis=-1)
    cos = np.repeat(np.cos(grid), 2, axis=-1)
    x1, x2 = x[..., ::2], x[..., 1::2]
    xs = np.stack([-x2, x1], axis=-1).reshape(x.shape)
    return x * cos + xs * sin


def _reference_np(x, mask, Wq, Wk, Wv, Wo):
    b, l, d = x.shape
    h, k_sz = H, D // H
    split = lambda t: t.reshape(b, l, h, k_sz).transpose(0, 2, 1, 3)
    q = split((x @ Wq) / np.sqrt(np.float32(d)))
    q = _rope_np(q)
    k = _rope_np(split(x @ Wk))
    v = split(x @ Wv)
    logits = np.einsum("bhik,bhjk->bhij", q, k) + mask
    m = logits.max(axis=-1, keepdims=True)
    p = np.exp(logits - m)
    a = p / p.sum(axis=-1, keepdims=True)
    y = np.einsum("bhij,bhjv->bhiv", a, v)
    y = y.transpose(0, 2, 1, 3).reshape(b, l, d)
    return (y @ Wo).astype(np.float32)


def _spectral_norm(w, iters=12):
    rng = np.random.default_rng(0)
    v = rng.standard_normal(w.shape[1]).astype(np.float32)
    for _ in range(iters):
        u = w @ v
        u /= (np.linalg.norm(u) + 1e-30)
        v = w.T @ u
        nv = np.linalg.norm(v)
        v /= (nv + 1e-30)
    return float(nv)


def _host_consts():
    inv = 10000.0 ** (-np.arange(0, HD, 2, dtype=np.float32) / HD)
    grid = np.arange(L, dtype=np.float32)[None, :] * inv[:, None]   # [32, L]
    cos32 = np.cos(grid).astype(np.float32)
    sin32 = np.sin(grid).astype(np.float32)
    cos128 = np.ascontiguousarray(np.tile(cos32, (4, 1)))
    # srot rows r: +sin[r%32] for r%64 < 32, -sin[r%32] otherwise
    srot128 = np.ascontiguousarray(
        np.tile(np.concatenate([sin32, -sin32], axis=0), (2, 1)))
    tri = (np.arange(128)[None, :] >= np.arange(128)[:, None]).astype(np.float32)
    return cos128, srot128, np.ascontiguousarray(tri)


def _make_in_maps(x, Wq, Wk, Wv, Wo):
    import ml_dtypes
    bf16 = ml_dtypes.bfloat16

    cos128, srot128, mk4 = _host_consts()
    cos128 = cos128.astype(bf16)
    srot128 = srot128.astype(bf16)
    mk4 = mk4.astype(bf16)
    perm = np.concatenate([np.arange(0, 64, 2), np.arange(1, 64, 2)])
    Wq_s = (Wq / np.sqrt(np.float32(D))).astype(np.float32)

    def chunked(w):
        # [D, 256] -> [128, KC*256] with model-dim chunk kc in the free dim
        return np.ascontiguousarray(
            w.reshape(KC, 128, 256).transpose(1, 0, 2).reshape(128, KC * 256))

    in_maps = []
    for core in range(NCORES):
        bi, g = core // 4, core % 4
        # x feature-major, lc-major: [128, LC, KC, 512], d = kc*128 + p
        xT_b = np.ascontiguousarray(
            x[bi].T.reshape(KC, 128, LC, 512).transpose(1, 2, 0, 3)).astype(bf16)
        wq_c = np.empty((D, 256), np.float32)
        wk_c = np.empty((D, 256), np.float32)
        for hh in range(HPC):
            h_abs = g * HPC + hh
            wq_c[:, hh * 64:(hh + 1) * 64] = Wq_s[:, h_abs * 64:(h_abs + 1) * 64][:, perm]
            wk_c[:, hh * 64:(hh + 1) * 64] = Wk[:, h_abs * 64:(h_abs + 1) * 64][:, perm]
        wo_c = Wo[g * 256:(g + 1) * 256, :]        # [256, D]
        wo_c = np.ascontiguousarray(
            wo_c.reshape(2, 128, D).transpose(1, 0, 2).reshape(128, 2 * D))
        in_maps.append({
            "xb": xT_b,
            "wq": chunked(wq_c).astype(bf16),
            "wk": chunked(wk_c).astype(bf16),
            "wv": chunked(Wv[:, g * 256:(g + 1) * 256].astype(np.float32)).astype(bf16),
            "wo": wo_c.astype(bf16),
            "cs128": cos128, "sn128": srot128, "mk4": mk4,
        })
    return in_maps


def kernel(x, mask, Wq, Wk, Wv, Wo):
    from concourse.bass_utils import run_bass_kernel_spmd

    x = np.asarray(x, dtype=np.float32)
    mask = np.asarray(mask, dtype=np.float32)
    Wq = np.asarray(Wq, dtype=np.float32)
    Wk = np.asarray(Wk, dtype=np.float32)
    Wv = np.asarray(Wv, dtype=np.float32)
    Wo = np.asarray(Wo, dtype=np.float32)

    # classify the mask
    m = mask.reshape(L, L)
    tril = np.tril(np.ones((L, L), dtype=bool))
    visible = m > -1e6
    if np.array_equal(visible, tril) and not m[tril].any():
        causal = True
    elif not m.any():
        causal = False
    else:
        return _reference_np(x, mask, Wq, Wk, Wv, Wo)

    # exact logit bound (rope preserves pair norms): guards the
    # no-max-subtraction softmax, and enables the vector-engine Taylor-exp
    # path for a few chunk-3 groups when logits are provably tiny
    q = x.reshape(-1, D) @ (Wq / np.sqrt(np.float32(D)))
    k = x.reshape(-1, D) @ Wk
    qn = np.sqrt((q.reshape(-1, H, HD) ** 2).sum(-1)).max(0)
    kn = np.sqrt((k.reshape(-1, H, HD) ** 2).sum(-1)).max(0)
    bound = float((qn * kn).max())
    if bound > 60.0:
        return _reference_np(x, mask, Wq, Wk, Wv, Wo)
    # note: a vector-engine Taylor-exp offload for chunk-3 groups was tried
    # here and regressed: it overloads the DVE queue and delays PV
    taylor = False

    in_maps = _make_in_maps(x, Wq, Wk, Wv, Wo)
    nc = _get_nc(causal, taylor)
    res = run_bass_kernel_spmd(nc, in_maps, core_ids=list(range(NCORES)))

    out = np.empty((B, L, D), dtype=np.float32)
    for bi in range(B):
        acc = res.results[bi * 4]["outT"].astype(np.float32)
        for g in range(1, 4):
            acc += res.results[bi * 4 + g]["outT"].astype(np.float32)
        out[bi] = acc.T
    return out


# revision 35
# speedup vs baseline: 1.1866x; 1.0006x over previous
"""Multi-head attention (16 heads, RoPE, causal) for Trainium2, 8 NeuronCores.

Sharding: data-parallel over batch (2) x tensor-parallel over head groups (4),
one (batch, head-group-of-4) pair per core. Each core computes its 4 heads'
attention feature-major (transposed) and a partial output projection
outT = Wo_slice^T @ Y^T [1024, 2048] in fp16; the host sums the 4 partials per
batch and transposes back.

Key structure (v3, pipelined):
  - Everything feature-major: Q^T/K^T [256, 2048] so the S^T (j-on-partition)
    matmuls need no on-chip transposes anywhere.
  - S^T for the two heads of a pair (rows 0:64 / 64:128 of a 128-row tile)
    is emitted interleaved so the K=64 matmuls run CONCURRENTLY in disjoint
    PE row-groups (tile_position auto-derived from base partition).
  - Projections (next l-chunk), V tiles, and the output projection of the
    previous chunk are interleaved as filler tensor work inside the attention
    jp-loop, overlapping with exp() on the scalar engine.
  - ~28 junk warm-up matmuls at t=0 keep the PE HAM clock-gate warm so the
    DMA-gated prologue projections run at 2.4 GHz instead of 1.2.
  - Output partials in fp16; RoPE tables built on-chip from 32-row cos/sin;
    input DMAs split across both HWDGE queues (sync + scalar).
  - softmax denominator comes from a ones-column appended to V (O_aug row 64)
    so no partition reductions; exp() has no max-subtraction (logits tiny for
    this problem family; host-side spectral bound checks and falls back).
"""

import sys

sys.path.insert(0, "/opt/trn_rl_repo")
sys.path.insert(0, "/root/.axon_site")

import numpy as np

B, L, D = 2, 2048, 1024
H = 16                  # total heads
HD = 64                 # head dim
HPC = 4                 # heads per core
NCORES = 8
NT = 2                  # head pairs per core (128-row tiles of Q^T/K^T/Y^T)
LC = L // 512           # 512-wide l chunks
KC = D // 128           # 128-deep contraction chunks over model dim
LT = L // 128           # 128-row l tiles

_cache = {}


def _build_nc(causal: bool, taylor: bool = False):
    import contextlib

    import concourse.bass as bass
    import concourse.tile as tile
    from concourse import bacc, mybir

    F32 = mybir.dt.float32
    F16 = mybir.dt.float16
    BF16 = mybir.dt.bfloat16
    EXP = mybir.ActivationFunctionType.Exp

    nc = bacc.Bacc("TRN2", target_bir_lowering=False, debug=False, num_devices=NCORES)

    # lc-major x layout: for a fixed (partition, lc) the (kc, 512) block is
    # 8KB contiguous in DRAM, so the per-lc DMAs move big lines (the kc-major
    # layout produced 1KB lines and halved effective DMA bandwidth).
    xb = nc.dram_tensor("xb", [128, LC, KC, 512], BF16, kind="ExternalInput")
    wq = nc.dram_tensor("wq", [128, KC * 256], BF16, kind="ExternalInput")
    wk = nc.dram_tensor("wk", [128, KC * 256], BF16, kind="ExternalInput")
    wv = nc.dram_tensor("wv", [128, KC * 256], BF16, kind="ExternalInput")
    wo = nc.dram_tensor("wo", [128, 2 * D], BF16, kind="ExternalInput")
    cs128 = nc.dram_tensor("cs128", [128, L], BF16, kind="ExternalInput")
    sn128 = nc.dram_tensor("sn128", [128, L], BF16, kind="ExternalInput")
    mk4 = nc.dram_tensor("mk4", [128, 128], BF16, kind="ExternalInput")
    outT = nc.dram_tensor("outT", [D, L], F16, kind="ExternalOutput")

    with tile.TileContext(nc) as tc, \
         nc.allow_low_precision(reason="bf16 matmul pipeline by design"), \
         contextlib.ExitStack() as ctx:
        p_w = ctx.enter_context(tc.tile_pool(name="p_w", bufs=3))
        p_wo = ctx.enter_context(tc.tile_pool(name="p_wo", bufs=1))
        p_const = ctx.enter_context(tc.tile_pool(name="p_const", bufs=4))
        p_x = ctx.enter_context(tc.tile_pool(name="p_x", bufs=5))
        p_junk = ctx.enter_context(tc.tile_pool(name="p_junk", bufs=1))
        p_qt = ctx.enter_context(tc.tile_pool(name="p_qt", bufs=2))
        p_kt = ctx.enter_context(tc.tile_pool(name="p_kt", bufs=2))
        p_yt = ctx.enter_context(tc.tile_pool(name="p_yt", bufs=2))
        p_v = ctx.enter_context(tc.tile_pool(name="p_v", bufs=16))
        p_pt = ctx.enter_context(tc.tile_pool(name="p_pt", bufs=6))
        p_tmp = ctx.enter_context(tc.tile_pool(name="p_tmp", bufs=4))
        p_z = ctx.enter_context(tc.tile_pool(name="p_z", bufs=8))
        p_oc = ctx.enter_context(tc.tile_pool(name="p_oc", bufs=4))
        pp = ctx.enter_context(tc.tile_pool(name="pp", bufs=2, space="PSUM"))
        pst = ctx.enter_context(tc.tile_pool(name="pst", bufs=2, space="PSUM"))
        pso = ctx.enter_context(tc.tile_pool(name="pso", bufs=2, space="PSUM"))

        # ---- PE warm-up + exp-table preload while DMAs run ----------
        # ~28 junk matmuls span the first ~8us so the HAM clock-gate is
        # warm (2.4 GHz) when the real, DMA-gated projections start.
        # The first 8 warm-up matmuls also ZERO all four pst banks: the
        # diagonal-trimmed S^T matmuls leave sub-regions of st unwritten,
        # and exp() of uninitialized PSUM garbage can overflow to inf (then
        # inf*0 = NaN in the causal mask multiply). The remaining short-N
        # matmuls just keep the PE HAM clock-gate warm (~10us span) until
        # the DMA-gated real projections start.
        junk = p_junk.tile([128, 512], BF16, tag="junk")
        nc.vector.memset(junk[:, :], 0.0)
        jps = [pst.tile([128, 1024], F32, tag="st", name=f"warm{i}")
               for i in range(2)]
        for r in range(8):
            t = jps[(r // 2) % 2]
            half = slice(0, 512) if r % 2 == 0 else slice(512, 1024)
            nc.tensor.matmul(t[:, half], junk[:, 0:128], junk[:, :],
                             start=True, stop=True)
        for r in range(48):
            nc.tensor.matmul(jps[r % 2][:, 0:128], junk[:, 0:128],
                             junk[:, 0:128], start=True, stop=True)

        # ---- input DMAs (split across the two HWDGE queues) ---------
        # sync queue: lc=0 x slices first (they gate the first projections),
        # then the cos table (first used by the rope muls a little later).
        xs_a = p_x.tile([128, 2, 512], BF16, tag="x", name="xa")
        nc.sync.dma_start(out=xs_a[:, :, :], in_=xb.ap()[:, 0, 0:2, :])
        xs_b = p_x.tile([128, 6, 512], BF16, tag="x", name="xb")
        nc.sync.dma_start(out=xs_b[:, :, :], in_=xb.ap()[:, 0, 2:8, :])
        cos_t = p_const.tile([128, L], BF16, tag="const", name="cos")
        nc.sync.dma_start(out=cos_t[:, :], in_=cs128.ap())
        xs_r = []
        for lc in range(1, LC):
            t = p_x.tile([128, KC, 512], BF16, tag="x", name=f"x{lc}")
            nc.sync.dma_start(out=t[:, :, :], in_=xb.ap()[:, lc, :, :])
            xs_r.append(t)

        def x_ap(kc, col0, ncol):
            # [128, ncol] slice of x at model chunk kc, columns col0:col0+ncol
            lc, o = col0 // 512, col0 % 512
            assert o + ncol <= 512
            if lc == 0:
                t = xs_a if kc < 2 else xs_b
                k = kc if kc < 2 else kc - 2
            else:
                t, k = xs_r[lc - 1], kc
            return t[:, k, o:o + ncol]

        # scalar queue: weights + srot table, in order of first use
        wq_sb = p_w.tile([128, KC, 256], BF16, tag="w", name="wq")
        nc.scalar.dma_start(out=wq_sb[:, :, :], in_=wq.ap())
        wk_sb = p_w.tile([128, KC, 256], BF16, tag="w", name="wk")
        nc.scalar.dma_start(out=wk_sb[:, :, :], in_=wk.ap())
        srot_t = p_const.tile([128, L], BF16, tag="const", name="srot")
        nc.scalar.dma_start(out=srot_t[:, :], in_=sn128.ap())
        wv_sb = p_w.tile([128, KC, 256], BF16, tag="w", name="wv")
        nc.scalar.dma_start(out=wv_sb[:, :, :], in_=wv.ap())
        mk_t = p_const.tile([128, 128], BF16, tag="tri")
        nc.scalar.dma_start(out=mk_t[:, :], in_=mk4.ap())
        wo_sb = p_wo.tile([128, 2, D], BF16, tag="wo")
        nc.scalar.dma_start(out=wo_sb[:, :, :], in_=wo.ap())

        # exp-table preload (~2.7us) overlaps the DMA flow
        dz = p_z.tile([1, 8], F32, tag="dz")
        nc.vector.memset(dz[:, :], 0.0)
        dz2 = p_z.tile([1, 8], F32, tag="dz2")
        nc.scalar.activation(dz2[0:1, :], dz[0:1, :], EXP)

        qt_sb = [p_qt.tile([128, L], BF16, tag="qt", name=f"qt{i}") for i in range(NT)]
        kt_sb = [p_kt.tile([128, L], BF16, tag="kt", name=f"kt{i}") for i in range(NT)]
        yt_sb = [p_yt.tile([128, L], BF16, tag="yt", name=f"yt{i}") for i in range(NT)]
        v_sb = [p_v.tile([128, HPC, 65], BF16, tag="vaug", name=f"vaug{i}")
                for i in range(LT)]

        # ---- unit emitters ------------------------------------------
        # rope: evacuate proj psum to bf16, then rotate-pairs. The four
        # narrow rotate muls go to the (otherwise idle) gpsimd engine for
        # steady-state units; prologue units keep them on the faster DVE
        # since they gate the start of attention.
        def rope_evac(ps, trg, lc, evac_act, narrows_gps):
            sl = slice(lc * 512, (lc + 1) * 512)
            qraw = p_tmp.tile([128, 512], BF16, tag="qraw")
            if evac_act:
                nc.scalar.copy(qraw[:, :], ps[:, :])
            else:
                nc.vector.tensor_copy(qraw[:, :], ps[:, :])
            eng = nc.gpsimd if narrows_gps else nc.vector
            tmp = p_tmp.tile([128, 512], BF16, tag="tmp")
            for hh in range(2):
                b0 = hh * 64
                eng.tensor_mul(tmp[b0:b0 + 32, :], qraw[b0 + 32:b0 + 64, :],
                               srot_t[b0 + 32:b0 + 64, sl])
                eng.tensor_mul(tmp[b0 + 32:b0 + 64, :], qraw[b0:b0 + 32, :],
                               srot_t[b0:b0 + 32, sl])
            nc.vector.tensor_mul(trg[:, sl], qraw[:, :], cos_t[:, sl])
            nc.vector.tensor_add(trg[:, sl], trg[:, sl], tmp[:, :])

        def proj_unit(w_sb, trg, nt, lc, evac_act=False, narrows_gps=False):
            ps = pp.tile([128, 512], F32, tag="pp")
            for kc in range(KC):
                nc.tensor.matmul(
                    ps[:, :], w_sb[:, kc, nt * 128:(nt + 1) * 128],
                    x_ap(kc, lc * 512, 512),
                    start=(kc == 0), stop=(kc == KC - 1))
            rope_evac(ps, trg, lc, evac_act, narrows_gps)

        def v_unit(lt, evac_act=False):
            ps = pp.tile([128, 256], F32, tag="pp")
            for kc in range(KC):
                nc.tensor.matmul(
                    ps[:, :], x_ap(kc, lt * 128, 128),
                    wv_sb[:, kc, :], start=(kc == 0), stop=(kc == KC - 1))
            va = v_sb[lt]
            nc.vector.memset(va[:, :, 64:65], 1.0)
            copy = nc.scalar.copy if evac_act else nc.vector.tensor_copy
            copy(va[:, :, 0:64], ps[:, :].rearrange("p (h v) -> p h v", h=HPC))

        def oproj_unit(c, ot, use_act):
            csl = slice(c * 512, (c + 1) * 512)
            ps = pp.tile([128, 512], F32, tag="pp")
            for kc2 in range(2):
                nc.tensor.matmul(
                    ps[:, :], wo_sb[:, kc2, ot * 128:(ot + 1) * 128],
                    yt_sb[kc2][:, csl], start=(kc2 == 0), stop=(kc2 == 1))
            oc = p_oc.tile([128, 512], F16, tag="oc")
            if use_act:
                nc.scalar.copy(oc[:, :], ps[:, :])
            else:
                nc.vector.tensor_copy(oc[:, :], ps[:, :])
            nc.sync.dma_start(
                out=outT.ap()[ot * 128:(ot + 1) * 128, csl], in_=oc[:, :])

        # ---- attention ----------------------------------------------
        def trim(j, c):
            k = j - 4 * c
            return 128 * k if (causal and k >= 0) else 0

        MULT = mybir.AluOpType.mult
        ADD = mybir.AluOpType.add

        def taylor_exp(pt, st):
            # exp(x) ~ 1 + x(1 + x/2(1 + x/3)) on the vector engine, used
            # for a few chunk-3 groups where the scalar engine saturates.
            # Valid because the host checked max|logit| is small.
            cb = p_tmp.tile([128, 1024], BF16, tag="tb")
            nc.vector.tensor_copy(cb[:, :], st[:, :])
            a = p_tmp.tile([128, 1024], BF16, tag="ta")
            nc.vector.tensor_scalar(a[:, :], cb[:, :], 1.0 / 3.0, 1.0, MULT, ADD)
            b = p_tmp.tile([128, 1024], BF16, tag="ta")
            nc.vector.scalar_tensor_tensor(b[:, :], a[:, :], 0.5, cb[:, :], MULT, MULT)
            c2 = p_tmp.tile([128, 1024], BF16, tag="ta")
            nc.vector.scalar_tensor_tensor(c2[:, :], b[:, :], 1.0, cb[:, :], ADD, MULT)
            nc.vector.tensor_scalar(pt[:, :], c2[:, :], 1.0, None, ADD)

        def emit_s(c, nt, jp):
            stA = pst.tile([128, 1024], F32, tag="st", name="stA")
            stB = pst.tile([128, 1024], F32, tag="st", name="stB")
            for s in range(2):
                j = 2 * jp + s
                # s=0 may trim its leading columns (exp starts there too);
                # s=1 writes its full 512 so exp never reads stale PSUM.
                t = trim(j, c) if s == 0 else 0
                for st, r0 in ((stA, 0), (stB, 64)):
                    nc.tensor.matmul(
                        st[:, s * 512 + t:(s + 1) * 512],
                        kt_sb[nt][r0:r0 + 64, j * 128:(j + 1) * 128],
                        qt_sb[nt][r0:r0 + 64, c * 512 + t:(c + 1) * 512],
                        start=True, stop=True)
            t0 = trim(2 * jp, c)
            pts = []
            for i, st in enumerate((stA, stB)):
                pt = p_pt.tile([128, 1024], BF16, tag="pt")
                if taylor and i == 1 and c == LC - 1 and jp in (1, 3, 5):
                    taylor_exp(pt, st)      # non-diagonal groups only (t0=0)
                else:
                    nc.scalar.activation(pt[:, t0:], st[:, t0:], EXP)
                pts.append(pt)
            if causal:
                for s in range(2):
                    k = 2 * jp + s - 4 * c
                    if k >= 0:
                        sl = slice(s * 512 + 128 * k, s * 512 + 128 * (k + 1))
                        for pt in pts:
                            nc.vector.tensor_mul(pt[:, sl], pt[:, sl], mk_t[:, :])
            return pts

        def emit_pv(c, nt, jp, pts, oaugs, jmax, heads=(0, 1)):
            for s in range(2):
                j = 2 * jp + s
                t = trim(j, c)
                for i in heads:
                    pt, oaug = pts[i], oaugs[i]
                    h = 2 * nt + i
                    nc.tensor.matmul(
                        oaug[:, t:512], v_sb[j][:, h, :],
                        pt[:, s * 512 + t:(s + 1) * 512],
                        start=(j == 0), stop=(j == jmax))

        def emit_norm(c, nt, oaugs, heads=(0, 1)):
            csl = slice(c * 512, (c + 1) * 512)
            for i in heads:
                oaug = oaugs[i]
                r0 = i * 64
                # stage z in SBUF first: reciprocal_approx_fast is a
                # multi-pass custom DVE op; feeding it PSUM directly has
                # produced corrupted single columns on hardware.
                zs = p_z.tile([1, 512], F32, tag="zs")
                nc.vector.tensor_copy(zs[0:1, :], oaug[64:65, :])
                zrow = p_z.tile([1, 512], F32, tag="zrow")
                nc.vector.reciprocal_approx_fast(zrow[0:1, :], zs[0:1, :])
                zb = p_z.tile([64, 512], F32, tag="zb")
                nc.gpsimd.partition_broadcast(zb[:, :], zrow[0:1, :])
                nc.vector.tensor_mul(yt_sb[nt][r0:r0 + 64, csl],
                                     oaug[0:64, :], zb[:, :])

        # ---- prologue: first chunk of projections + first V tiles ---
        # (dense mode emits all V tiles up front: PV(c=0) touches every j)
        # evac copies on the idle scalar engine, rotate muls on DVE: the
        # vector-engine chain here gates the start of attention.
        for nt in range(NT):
            proj_unit(wq_sb, qt_sb[nt], nt, 0, evac_act=True, narrows_gps=False)
            proj_unit(wk_sb, kt_sb[nt], nt, 0, evac_act=True, narrows_gps=False)
        for lt in range(4 if causal else LT):
            v_unit(lt, evac_act=True)

        # ---- main pipelined loop ------------------------------------
        for c in range(LC):
            jmax = 4 * c + 3 if causal else LT - 1
            npj = (jmax + 1) // 2            # jp count per pair

            # filler units to interleave into this chunk's attention.
            # out-projections are deferred toward the last chunk, which is
            # otherwise exp-bound (its slots have spare tensor time).
            fills = []
            if c < LC - 1:
                for nt in range(NT):
                    fills.append(lambda nt=nt: proj_unit(wq_sb, qt_sb[nt], nt, c + 1,
                                                         evac_act=True))
                    fills.append(lambda nt=nt: proj_unit(wk_sb, kt_sb[nt], nt, c + 1,
                                                         evac_act=True))
                if causal:
                    for lt in range(4 * (c + 1), 4 * (c + 2)):
                        fills.append(lambda lt=lt: v_unit(lt, evac_act=True))
            for cp in {2: [0], 3: [1, 2]}.get(c, []):
                for ot in range(8):
                    fills.append(lambda ot=ot, cp=cp: oproj_unit(cp, ot, False))

            nslots = 2 * npj
            state = {"fi": 0}

            def emit_fills(progress, fills=fills, state=state, nslots=nslots):
                want = min(len(fills), (len(fills) * progress) // (2 * nslots))
                while state["fi"] < want:
                    fills[state["fi"]]()
                    state["fi"] += 1

            slot = 0
            for nt in range(NT):
                oaugs = [pso.tile([65, 512], F32, tag="oaug", name=f"oaug{i}")
                         for i in range(2)]
                lag = []
                for jp in range(npj):
                    # fills first: their psum-evac copies land on the ACT/DVE
                    # queues ahead of this slot's exps, releasing pp sooner
                    emit_fills(2 * slot + 1)
                    pts = emit_s(c, nt, jp)
                    if lag:
                        emit_pv(c, nt, *lag.pop(0), oaugs, jmax)
                    slot += 1
                    emit_fills(2 * slot)
                    lag.append((jp, pts))
                # tail: per-head PV + norm interleaved, so head A's norm
                # chain (DVE/gpsimd) overlaps head B's final PV matmuls
                for jp, pts in lag:
                    emit_pv(c, nt, jp, pts, oaugs, jmax, heads=(0,))
                    emit_norm(c, nt, oaugs, heads=(0,))
                    emit_pv(c, nt, jp, pts, oaugs, jmax, heads=(1,))
                    emit_norm(c, nt, oaugs, heads=(1,))
            emit_fills(2 * nslots)

        # keep the PE clock-gate warm through the final norm chain (the
        # ~4.5us gap otherwise re-throttles it and the tail out-projection
        # then runs at half clock)
        warm3 = pst.tile([128, 1024], F32, tag="st", name="warm3")
        for r in range(16):
            nc.tensor.matmul(warm3[:, 0:128], junk[:, 0:128], junk[:, 0:128],
                             start=True, stop=True)

        # ---- tail: output projection of the last chunk --------------
        for ot in range(8):
            oproj_unit(LC - 1, ot, ot % 2 == 1)

    nc.compile()
    return nc


def _get_nc(causal: bool, taylor: bool = False):
    key = ("causal" if causal else "dense") + ("_taylor" if taylor else "")
    if key not in _cache:
        _cache[key] = _build_nc(causal, taylor)
    return _cache[key]


def _rope_np(x):
    d, s = x.shape[-1], x.shape[-2]
    ts = np.arange(0, d, 2, dtype=np.float32)
    inv = 10000.0 ** (-ts / d)
    grid = np.arange(s, dtype=np.float32)[:, None] * inv[None, :]
    sin = np.repeat(np.sin(grid), 2, axis=-1)
    cos = np.repeat(np.cos(grid), 2, axis=-1)
    x1, x2 = x[..., ::2], x[..., 1::2]
    xs = np.stack([-x2, x1], axis=-1).reshape(x.shape)
    return x * cos + xs * sin


def _reference_np(x, mask, Wq, Wk, Wv, Wo):
    b, l, d = x.shape
    h, k_sz = H, D // H
    split = lambda t: t.reshape(b, l, h, k_sz).transpose(0, 2, 1, 3)
    q = split((x @ Wq) / np.sqrt(np.float32(d)))
    q = _rope_np(q)
    k = _rope_np(split(x @ Wk))
    v = split(x @ Wv)
    logits = np.einsum("bhik,bhjk->bhij", q, k) + mask
    m = logits.max(axis=-1, keepdims=True)
    p = np.exp(logits - m)
    a = p / p.sum(axis=-1, keepdims=True)
    y = np.einsum("bhij,bhjv->bhiv", a, v)
    y = y.transpose(0, 2, 1, 3).reshape(b, l, d)
    return (y @ Wo).astype(np.float32)


def _spectral_norm(w, iters=12):
    rng = np.random.default_rng(0)
    v = rng.standard_normal(w.shape[1]).astype(np.float32)
    for _ in range(iters):
        u = w @ v
        u /= (np.linalg.norm(u) + 1e-30)
        v = w.T @ u
        nv = np.linalg.norm(v)
        v /= (nv + 1e-30)
    return float(nv)


def _host_consts():
    inv = 10000.0 ** (-np.arange(0, HD, 2, dtype=np.float32) / HD)
    grid = np.arange(L, dtype=np.float32)[None, :] * inv[:, None]   # [32, L]
    cos32 = np.cos(grid).astype(np.float32)
    sin32 = np.sin(grid).astype(np.float32)
    cos128 = np.ascontiguousarray(np.tile(cos32, (4, 1)))
    # srot rows r: +sin[r%32] for r%64 < 32, -sin[r%32] otherwise
    srot128 = np.ascontiguousarray(
        np.tile(np.concatenate([sin32, -sin32], axis=0), (2, 1)))
    tri = (np.arange(128)[None, :] >= np.arange(128)[:, None]).astype(np.float32)
    return cos128, srot128, np.ascontiguousarray(tri)


def _make_in_maps(x, Wq, Wk, Wv, Wo):
    import ml_dtypes
    bf16 = ml_dtypes.bfloat16

    cos128, srot128, mk4 = _host_consts()
    cos128 = cos128.astype(bf16)
    srot128 = srot128.astype(bf16)
    mk4 = mk4.astype(bf16)
    perm = np.concatenate([np.arange(0, 64, 2), np.arange(1, 64, 2)])
    Wq_s = (Wq / np.sqrt(np.float32(D))).astype(np.float32)

    def chunked(w):
        # [D, 256] -> [128, KC*256] with model-dim chunk kc in the free dim
        return np.ascontiguousarray(
            w.reshape(KC, 128, 256).transpose(1, 0, 2).reshape(128, KC * 256))

    in_maps = []
    for core in range(NCORES):
        bi, g = core // 4, core % 4
        # x feature-major, lc-major: [128, LC, KC, 512], d = kc*128 + p
        xT_b = np.ascontiguousarray(
            x[bi].T.reshape(KC, 128, LC, 512).transpose(1, 2, 0, 3)).astype(bf16)
        wq_c = np.empty((D, 256), np.float32)
        wk_c = np.empty((D, 256), np.float32)
        for hh in range(HPC):
            h_abs = g * HPC + hh
            wq_c[:, hh * 64:(hh + 1) * 64] = Wq_s[:, h_abs * 64:(h_abs + 1) * 64][:, perm]
            wk_c[:, hh * 64:(hh + 1) * 64] = Wk[:, h_abs * 64:(h_abs + 1) * 64][:, perm]
        wo_c = Wo[g * 256:(g + 1) * 256, :]        # [256, D]
        wo_c = np.ascontiguousarray(
            wo_c.reshape(2, 128, D).transpose(1, 0, 2).reshape(128, 2 * D))
        in_maps.append({
            "xb": xT_b,
            "wq": chunked(wq_c).astype(bf16),
            "wk": chunked(wk_c).astype(bf16),
            "wv": chunked(Wv[:, g * 256:(g + 1) * 256].astype(np.float32)).astype(bf16),
            "wo": wo_c.astype(bf16),
            "cs128": cos128, "sn128": srot128, "mk4": mk4,
        })
    return in_maps


def kernel(x, mask, Wq, Wk, Wv, Wo):
    from concourse.bass_utils import run_bass_kernel_spmd

    x = np.asarray(x, dtype=np.float32)
    mask = np.asarray(mask, dtype=np.float32)
    Wq = np.asarray(Wq, dtype=np.float32)
    Wk = np.asarray(Wk, dtype=np.float32)
    Wv = np.asarray(Wv, dtype=np.float32)
    Wo = np.asarray(Wo, dtype=np.float32)

    # classify the mask
    m = mask.reshape(L, L)
    tril = np.tril(np.ones((L, L), dtype=bool))
    visible = m > -1e6
    if np.array_equal(visible, tril) and not m[tril].any():
        causal = True
    elif not m.any():
        causal = False
    else:
        return _reference_np(x, mask, Wq, Wk, Wv, Wo)

    # exact logit bound (rope preserves pair norms): guards the
    # no-max-subtraction softmax, and enables the vector-engine Taylor-exp
    # path for a few chunk-3 groups when logits are provably tiny
    q = x.reshape(-1, D) @ (Wq / np.sqrt(np.float32(D)))
    k = x.reshape(-1, D) @ Wk
    qn = np.sqrt((q.reshape(-1, H, HD) ** 2).sum(-1)).max(0)
    kn = np.sqrt((k.reshape(-1, H, HD) ** 2).sum(-1)).max(0)
    bound = float((qn * kn).max())
    if bound > 60.0:
        return _reference_np(x, mask, Wq, Wk, Wv, Wo)
    # note: a vector-engine Taylor-exp offload for chunk-3 groups was tried
    # here and regressed: it overloads the DVE queue and delays PV
    taylor = False

    in_maps = _make_in_maps(x, Wq, Wk, Wv, Wo)
    nc = _get_nc(causal, taylor)
    res = run_bass_kernel_spmd(nc, in_maps, core_ids=list(range(NCORES)))

    out = np.empty((B, L, D), dtype=np.float32)
    for bi in range(B):
        acc = res.results[bi * 4]["outT"].astype(np.float32)
        for g in range(1, 4):
            acc += res.results[bi * 4 + g]["outT"].astype(np.float32)
        out[bi] = acc.T
    return out
